# revision 36
# baseline (speedup 1.0000x reference)
"""GAT + global-max-pool + LSTM + Linear kernel for Trainium2 (8 NeuronCores), v2.

Sharding: data-parallel over batch B=8 -> one sequence b per core.

GAT reformulation (exact, per graph g, head h):
  softmax over in-edges of dst n is invariant to any per-column scale, so
  divide the attention matrix by u[m]*v[n] (u=exp(a_s), v=exp(a_d)):
    A~[m,n] = max(rho[m]*y[n], 1) * C[m,n]
  with rho = exp(-0.8*a_s), y = exp(-0.8*a_d), C = edge-count mask.
  The row factor u[m] folds into the aggregation lhsT (xpu = xp*u, u in the
  33rd denominator slot), the column factor v[n] cancels in num/den.

  Per-tile routes (tile = [128 src x 1024 dst], 8 per (g,h)):
   R4 : R = ACT-Relu(yB*rho - 1) ; RC = R*C (DVE TT) ; PE aggregates RC and C
   R4G: same but RC on GpSimd
   R1 : A~ = DVE TS max(yB*rho, 1) ; tA = A~*C (DVE TT) ; PE aggregates tA
  Epilogue uses max_n relu(x) = relu(max_n x) to pool before bias+relu.
"""

import numpy as np

import concourse.bacc as bacc
import concourse.bass as bass
import concourse.mybir as mybir
import concourse.tile as tile
from concourse.bass_utils import run_bass_kernel_spmd

B, T, N, F_IN = 8, 16, 1000, 16
H, D = 4, 32
HD = H * D          # 128
HL = 64
OUT = 8
NPAD = 1024
NBLK = 8
G = T

FP = mybir.dt.float32
BF = mybir.dt.bfloat16
AX = mybir.AxisListType
AF = mybir.ActivationFunctionType
OPS = mybir.AluOpType

# route per (h, J): 4=R4 (DVE mask), 5=R4G (GpSimd mask), 1=R1 (DVE TS+TT)
ROUTE = [
    [4, 2, 2, 1, 2, 2, 4, 2],
    [2, 4, 2, 2, 1, 2, 2, 4],
    [4, 2, 2, 1, 2, 2, 4, 2],
    [2, 4, 2, 2, 1, 2, 2, 4],
]

_CACHE = {}


def _build_nc():
    nc = bacc.Bacc("TRN2", target_bir_lowering=False, debug=False)

    # ---- DRAM I/O ----
    d_xt = nc.dram_tensor("x_t", [F_IN, G * NPAD], BF, kind="ExternalInput").ap()
    d_wgat = nc.dram_tensor("w_gat", [F_IN, HD], BF, kind="ExternalInput").ap()
    d_was = nc.dram_tensor("w_as", [F_IN, H], BF, kind="ExternalInput").ap()
    d_wad = nc.dram_tensor("w_ad", [F_IN, H], BF, kind="ExternalInput").ap()
    d_cnt = nc.dram_tensor("cntmask", [128, NBLK * NPAD], BF, kind="ExternalInput").ap()
    d_ones = nc.dram_tensor("ones65", [65, 128], BF, kind="ExternalInput").ap()
    d_bgat = nc.dram_tensor("b_gat", [32, H], FP, kind="ExternalInput").ap()
    d_wih = nc.dram_tensor("wih_t", [HD, 4 * HL], FP, kind="ExternalInput").ap()
    d_whh = nc.dram_tensor("whh_t", [HL, 4 * HL], FP, kind="ExternalInput").ap()
    d_bls = nc.dram_tensor("b_lstm", [HL, 4], FP, kind="ExternalInput").ap()
    d_wclf = nc.dram_tensor("wclf_t", [HL, OUT], FP, kind="ExternalInput").ap()
    d_bclf = nc.dram_tensor("b_clf", [OUT, 1], FP, kind="ExternalInput").ap()
    d_y = nc.dram_tensor("y", [OUT, 1], FP, kind="ExternalOutput").ap()

    with tile.TileContext(nc) as tc:
        with (
            tc.tile_pool(name="const", bufs=1) as cpool,
            tc.tile_pool(name="stage", bufs=3) as spool,
            tc.tile_pool(name="edense", bufs=4) as epool,
            tc.tile_pool(name="small", bufs=2) as mpool,
            tc.tile_pool(name="lstm", bufs=2) as lpool,
            tc.tile_pool(name="ps_misc", bufs=2, space="PSUM") as ps_misc,
            tc.tile_pool(name="ps_out", bufs=3, space="PSUM") as ps_out,
            tc.tile_pool(name="ps_y", bufs=1, space="PSUM") as ps_y,
        ):
            # ---- constants ----
            c_xT = cpool.tile([F_IN, G * NPAD], BF, tag="xT")
            nc.sync.dma_start(c_xT[:], d_xt)
            c_wgat = cpool.tile([F_IN, HD], BF, tag="wgat")
            nc.sync.dma_start(c_wgat[:], d_wgat)
            c_was = cpool.tile([F_IN, H], BF, tag="was")
            nc.sync.dma_start(c_was[:], d_was)
            c_wad = cpool.tile([F_IN, H], BF, tag="wad")
            nc.sync.dma_start(c_wad[:], d_wad)
            c_cnt = cpool.tile([128, NBLK * NPAD], BF, tag="cnt")
            nc.sync.dma_start(c_cnt[:], d_cnt)
            c_onesB = cpool.tile([65, 128], BF, tag="onesB")
            nc.sync.dma_start(c_onesB[:], d_ones)
            c_bgat = cpool.tile([32, H], FP, tag="bgat")
            nc.sync.dma_start(c_bgat[:], d_bgat)
            c_wih = cpool.tile([HD, 4 * HL], FP, tag="wih")
            nc.sync.dma_start(c_wih[:], d_wih)
            c_whh = cpool.tile([HL, 4 * HL], FP, tag="whh")
            nc.sync.dma_start(c_whh[:], d_whh)
            c_bls = cpool.tile([HL, 4], FP, tag="bls")
            nc.sync.dma_start(c_bls[:], d_bls)
            c_wclf = cpool.tile([HL, OUT], FP, tag="wclf")
            nc.sync.dma_start(c_wclf[:], d_wclf)
            c_bclf = cpool.tile([OUT, 1], FP, tag="bclf")
            nc.sync.dma_start(c_bclf[:], d_bclf)

            c_neg1 = cpool.tile([128, 1], FP, tag="neg1")
            nc.vector.memset(c_neg1[:], -1.0)
            c_ones32 = cpool.tile([33, 32], FP, tag="ones32")
            nc.vector.memset(c_ones32[:], 1.0)
            c_pool = cpool.tile([HD, G], FP, tag="pooled")
            c_ph = []
            for h in range(H):
                ph_tile = cpool.tile([32, G], FP, tag=f"pool{h}")
                c_ph.append(ph_tile)

            hprev0 = lpool.tile([HL, 1], FP, tag="h0")
            cprev0 = lpool.tile([HL, 1], FP, tag="c0")
            nc.vector.memset(hprev0[:], 0.0)
            nc.vector.memset(cprev0[:], 0.0)
            LST = [hprev0, cprev0]

            def emit_lstm_step(t):
                # pooled col t -> c_pool, then one LSTM step (overlaps GAT)
                for h in range(H):
                    nc.sync.dma_start(
                        c_pool[h * 32:(h + 1) * 32, t:t + 1], c_ph[h][:, t:t + 1]
                    )
                hprev, cprev = LST
                tga = []
                for gate in range(4):
                    psg = ps_misc.tile([HL, 1], FP, tag="pm")
                    nc.tensor.matmul(
                        psg[:], c_wih[:, gate * HL:(gate + 1) * HL],
                        c_pool[:, t:t + 1], start=True, stop=False,
                    )
                    nc.tensor.matmul(
                        psg[:], c_whh[:, gate * HL:(gate + 1) * HL],
                        hprev[:], start=False, stop=True,
                    )
                    tgt = lpool.tile([HL, 1], FP, tag=f"tg{gate}")
                    sc = 1.0 if gate == 2 else 0.5
                    nc.scalar.activation(
                        tgt[:], psg[:], AF.Tanh,
                        bias=c_bls[:, gate:gate + 1], scale=sc,
                    )
                    tga.append(tgt)
                ti, tf, tg_, to = tga
                v1 = lpool.tile([HL, 1], FP, tag="v1")
                nc.vector.scalar_tensor_tensor(
                    v1[:], tf[:], 1.0, cprev[:], OPS.add, OPS.mult
                )
                v2 = lpool.tile([HL, 1], FP, tag="v2")
                nc.vector.scalar_tensor_tensor(
                    v2[:], ti[:], 1.0, tg_[:], OPS.add, OPS.mult
                )
                cnew = lpool.tile([HL, 1], FP, tag="c0")
                nc.vector.scalar_tensor_tensor(
                    cnew[:], v1[:], 0.5, v2[:], OPS.mult, OPS.add
                )
                tcn = lpool.tile([HL, 1], FP, tag="tcn")
                nc.scalar.activation(tcn[:], cnew[:], AF.Tanh, scale=0.5)
                hnew = lpool.tile([HL, 1], FP, tag="h0")
                nc.vector.scalar_tensor_tensor(
                    hnew[:], to[:], 1.0, tcn[:], OPS.add, OPS.mult
                )
                LST[0], LST[1] = hnew, cnew

            PENDING = [None]
            for g in range(G):
                xg = c_xT[:, g * NPAD:(g + 1) * NPAD]   # [16, 1024] bf16

                # ---- per-g stage: a_s cols (u, rho), a_d rows (y), xpu33 ----
                pS = ps_misc.tile([128, 4 * NBLK], FP, tag="pm")
                for J in range(NBLK):
                    nc.tensor.matmul(
                        pS[:, J * 4:(J + 1) * 4],
                        xg[:, J * 128:(J + 1) * 128], c_was[:],
                        start=True, stop=True,
                    )
                c_u = spool.tile([128, 4 * NBLK], FP, tag="ucols")
                nc.scalar.activation(c_u[:], pS[:], AF.Exp, scale=1.0)
                c_rho = spool.tile([128, 4 * NBLK], FP, tag="rhocols")
                nc.scalar.activation(c_rho[:], pS[:], AF.Exp, scale=-0.8)

                y4 = spool.tile([4, NPAD], BF, tag="y4")
                for half in range(2):
                    pAd = ps_misc.tile([4, 512], FP, tag="pm")
                    nc.tensor.matmul(
                        pAd[:],
                        c_wad[:], xg[:, half * 512:(half + 1) * 512],
                        start=True, stop=True,
                    )
                    nc.scalar.activation(
                        y4[:, half * 512:(half + 1) * 512], pAd[:],
                        AF.Exp, scale=-0.8,
                    )
                # per-head base-0 row tiles for the gpsimd broadcast
                yrows = []
                for h in range(H):
                    y1h = spool.tile([1, NPAD], BF, tag=f"y1_{h}")
                    nc.sync.dma_start(y1h[:], y4[h:h + 1, :])
                    yrows.append(y1h)

                # xpu33: [128, J*132 + h*33 + (0..31 feats, 32 = u)]
                xpu33 = spool.tile([128, NBLK * 132], BF, tag="xpu33")
                for J in range(NBLK):
                    pX = ps_misc.tile([128, HD], FP, tag="pm")
                    nc.tensor.matmul(
                        pX[:], xg[:, J * 128:(J + 1) * 128], c_wgat[:],
                        start=True, stop=True,
                    )
                    base = J * 132
                    for h in range(H):
                        if J % 2 == 0:
                            nc.vector.tensor_scalar(
                                xpu33[:, base + h * 33:base + h * 33 + 32],
                                pX[:, h * 32:(h + 1) * 32],
                                c_u[:, J * 4 + h:J * 4 + h + 1], 0.0,
                                OPS.mult, OPS.add,
                            )
                        else:
                            nc.scalar.activation(
                                xpu33[:, base + h * 33:base + h * 33 + 32],
                                pX[:, h * 32:(h + 1) * 32], AF.Copy,
                                scale=c_u[:, J * 4 + h:J * 4 + h + 1],
                            )
                    nc.vector.tensor_copy(
                        xpu33[:, base:base + 132].rearrange(
                            "p (h q) -> p h q", q=33
                        )[:, :, 32:33],
                        c_u[:, J * 4:(J + 1) * 4].rearrange("p (h q) -> p h q", q=1),
                    )

                # ---- hot loop, software-pipelined one phase ahead:
                # PE stream per phase: [C-matmuls(h), yB(h)] then [tA-matmuls(h-1)]
                # so dependent matmuls trail their producers by a full phase.
                def emit_front(h):
                    oph = ps_out.tile([33, NPAD], FP, tag="oph")
                    n_mm = sum(2 if ROUTE[h][J] == 4 else 1 for J in range(NBLK))
                    mm_i = 0
                    for J in range(NBLK):
                        if ROUTE[h][J] != 4:
                            continue
                        cslice = c_cnt[:, J * NPAD:(J + 1) * NPAD]
                        lhs = xpu33[:, J * 132 + h * 33:J * 132 + h * 33 + 33]
                        for half in range(2):
                            sl = slice(half * 512, (half + 1) * 512)
                            nc.tensor.matmul(
                                oph[:, sl], lhs, cslice[:, sl],
                                start=(mm_i == 0), stop=(mm_i == n_mm - 1),
                            )
                        mm_i += 1
                    yB = epool.tile([128, NPAD], BF, tag="yB16")
                    nc.gpsimd.partition_broadcast(yB[:], yrows[h][:])
                    return (g, h, oph, yB, mm_i, n_mm, xpu33, c_rho)

                def produce(ctx):
                    g_l, h, oph, yB, mm_i, n_mm, xpu33_l, c_rho_l = ctx
                    rhss = []
                    for J in range(NBLK):
                        rt = ROUTE[h][J]
                        rho_col = c_rho_l[:, J * 4 + h:J * 4 + h + 1]
                        cslice = c_cnt[:, J * NPAD:(J + 1) * NPAD]
                        if rt == 1:
                            tAh = epool.tile([128, NPAD], BF, tag="tAh")
                            nc.vector.tensor_scalar(
                                tAh[:], yB[:], rho_col, 1.0, OPS.mult, OPS.max
                            )
                            rhs = epool.tile([128, NPAD], BF, tag="tA")
                            nc.vector.tensor_tensor(rhs[:], tAh[:], cslice, OPS.mult)
                        elif rt == 2:
                            R = epool.tile([128, NPAD], BF, tag="R")
                            nc.scalar.activation(
                                R[:], yB[:], AF.Relu, bias=c_neg1[:], scale=rho_col
                            )
                            rhs = epool.tile([128, NPAD], BF, tag="tA2")
                            nc.vector.scalar_tensor_tensor(
                                rhs[:], R[:], 1.0, cslice, OPS.add, OPS.mult
                            )
                        else:
                            R = epool.tile([128, NPAD], BF, tag="R")
                            nc.scalar.activation(
                                R[:], yB[:], AF.Relu, bias=c_neg1[:], scale=rho_col
                            )
                            rhs = epool.tile([128, NPAD], BF, tag="RC")
                            nc.vector.tensor_tensor(rhs[:], R[:], cslice, OPS.mult)
                        rhss.append(rhs)
                    return ctx + (rhss,)

                def consume(ctx):
                    g_l, h, oph, yB, mm_i, n_mm, xpu33_l, c_rho_l, rhss = ctx
                    for J in range(NBLK):
                        lhs = xpu33_l[:, J * 132 + h * 33:J * 132 + h * 33 + 33]
                        rhs = rhss[J]
                        for half in range(2):
                            sl = slice(half * 512, (half + 1) * 512)
                            nc.tensor.matmul(
                                oph[:, sl], lhs, rhs[:, half * 512:(half + 1) * 512],
                                start=(mm_i == 0), stop=(mm_i == n_mm - 1),
                            )
                        mm_i += 1
                    # epilogue
                    denr = mpool.tile([33, NPAD], FP, tag="denr")
                    nc.scalar.copy(denr[32:33, :], oph[32:33, :])
                    den32 = mpool.tile([32, 32], FP, tag="den32")
                    nc.sync.dma_start(den32[:], denr[32:33, :])
                    rec32 = mpool.tile([32, 32], FP, tag="rec32")
                    nc.vector.reciprocal(rec32[:], den32[:])
                    rech = mpool.tile([1, NPAD], FP, tag="rech")
                    nc.sync.dma_start(rech[:], rec32[:])
                    oph16 = mpool.tile([32, NPAD], BF, tag="oph16")
                    nc.scalar.copy(oph16[:], oph[0:32, :])
                    od = mpool.tile([32, NPAD], BF, tag="od")
                    for half in range(2):
                        sl = slice(half * 512, (half + 1) * 512)
                        rb = ps_misc.tile([32, 512], FP, tag="pm")
                        nc.tensor.matmul(
                            rb[:], c_ones32[0:1, :], rech[:, sl],
                            start=True, stop=True,
                        )
                        nc.vector.tensor_tensor(
                            od[:, sl], oph16[:, sl], rb[:], OPS.mult
                        )
                    trout = mpool.tile([32, 1], FP, tag="trout")
                    nc.vector.tensor_reduce(trout[:], od[:, 0:N], AX.X, OPS.max)
                    nc.vector.tensor_scalar(
                        c_ph[h][:, g_l:g_l + 1], trout[:],
                        c_bgat[:, h:h + 1], 0.0, OPS.add, OPS.max,
                    )

                for h in range(H):
                    ctx = emit_front(h)
                    if PENDING[0] is not None:
                        consume(produce(PENDING[0]))
                    PENDING[0] = ctx
                if g >= 1:
                    emit_lstm_step(g - 1)

            if PENDING[0] is not None:
                consume(produce(PENDING[0]))
                PENDING[0] = None
            emit_lstm_step(G - 1)

            ps3 = ps_misc.tile([OUT, 1], FP, tag="pm")
            nc.tensor.matmul(ps3[:], c_wclf[:], LST[0][:], start=True, stop=True)
            ysb = lpool.tile([OUT, 1], FP, tag="ysb")
            nc.vector.tensor_tensor(ysb[:], ps3[:], c_bclf[:], OPS.add)
            nc.sync.dma_start(d_y, ysb[:])

    nc.compile()
    return nc


def _host_prep(inputs):
    x = np.asarray(inputs["x"], dtype=np.float32)
    ei = np.asarray(inputs["edge_index"])
    W_gat = np.asarray(inputs["W_gat"], dtype=np.float32)
    att_src = np.asarray(inputs["att_src"], dtype=np.float32)
    att_dst = np.asarray(inputs["att_dst"], dtype=np.float32)
    b_gat = np.asarray(inputs["b_gat"], dtype=np.float32)
    W_ih = np.asarray(inputs["W_ih"], dtype=np.float32)
    W_hh = np.asarray(inputs["W_hh"], dtype=np.float32)
    b_ih = np.asarray(inputs["b_ih"], dtype=np.float32)
    b_hh = np.asarray(inputs["b_hh"], dtype=np.float32)
    W_clf = np.asarray(inputs["W_clf"], dtype=np.float32)
    b_clf = np.asarray(inputs["b_clf"], dtype=np.float32)

    bf16 = mybir.dt.np(BF)

    Wr = W_gat.reshape(F_IN, H, D)
    W_as = np.einsum("fhd,hd->fh", Wr, att_src)
    W_ad = np.einsum("fhd,hd->fh", Wr, att_dst)

    src = ei[0].astype(np.int64)
    dst = ei[1].astype(np.int64)
    Cm = np.zeros((NPAD, NPAD), dtype=np.float32)
    np.add.at(Cm, (src, dst), 1.0)
    Cm[np.arange(N), np.arange(N)] += 1.0
    Cm[NPAD - 1, N:] = 1.0
    cntmask = (
        Cm.reshape(NBLK, 128, NPAD).transpose(1, 0, 2).reshape(128, NBLK * NPAD)
    ).astype(bf16)

    xpad = np.zeros((B, T, NPAD, F_IN), dtype=np.float32)
    xpad[:, :, :N, :] = x
    # [F, T*NPAD] per core
    xtcore = [
        np.ascontiguousarray(
            xpad[b].reshape(T * NPAD, F_IN).T
        ).astype(bf16)
        for b in range(B)
    ]

    b_gates = (b_ih + b_hh).astype(np.float32)
    bls = np.zeros((HL, 4), dtype=np.float32)
    bls[:, 0] = 0.5 * b_gates[0:64]
    bls[:, 1] = 0.5 * b_gates[64:128]
    bls[:, 2] = b_gates[128:192]
    bls[:, 3] = 0.5 * b_gates[192:256]

    common = {
        "w_gat": W_gat.astype(bf16),
        "w_as": W_as.astype(bf16),
        "w_ad": W_ad.astype(bf16),
        "cntmask": cntmask,
        "ones65": np.ones((65, 128), dtype=bf16),
        "b_gat": np.ascontiguousarray(b_gat.reshape(H, 32).T),
        "wih_t": np.ascontiguousarray(W_ih.T),
        "whh_t": np.ascontiguousarray(0.5 * W_hh.T),
        "b_lstm": bls,
        "wclf_t": np.ascontiguousarray(0.5 * W_clf.T),
        "b_clf": b_clf.reshape(OUT, 1),
    }
    in_maps = []
    for b in range(B):
        m = dict(common)
        m["x_t"] = xtcore[b]
        in_maps.append(m)
    return in_maps


def kernel(**inputs):
    if "nc" not in _CACHE:
        _CACHE["nc"] = _build_nc()
    nc = _CACHE["nc"]
    in_maps = _host_prep(inputs)
    res = run_bass_kernel_spmd(nc, in_maps, core_ids=list(range(B)))
    y = np.stack([r["y"][:, 0] for r in res.results], axis=0)
    return y.astype(np.float32)


if __name__ == "__main__":
    import reference as R

    inp = R.setup_inputs()
    inp = {k: np.asarray(v) for k, v in inp.items()}
    out = kernel(**inp)
    print(out)


# revision 37
# speedup vs baseline: 1.0650x; 1.0650x over previous
"""GAT + global-max-pool + LSTM + Linear kernel for Trainium2 (8 NeuronCores), v2.

Sharding: data-parallel over batch B=8 -> one sequence b per core.

GAT reformulation (exact, per graph g, head h):
  softmax over in-edges of dst n is invariant to any per-column scale, so
  divide the attention matrix by u[m]*v[n] (u=exp(a_s), v=exp(a_d)):
    A~[m,n] = max(rho[m]*y[n], 1) * C[m,n]
  with rho = exp(-0.8*a_s), y = exp(-0.8*a_d), C = edge-count mask.
  The row factor u[m] folds into the aggregation lhsT (xpu = xp*u, u in the
  33rd denominator slot), the column factor v[n] cancels in num/den.

  Per-tile routes (tile = [128 src x 1024 dst], 8 per (g,h)):
   R4 : R = ACT-Relu(yB*rho - 1) ; RC = R*C (DVE TT) ; PE aggregates RC and C
   R4G: same but RC on GpSimd
   R1 : A~ = DVE TS max(yB*rho, 1) ; tA = A~*C (DVE TT) ; PE aggregates tA
  Epilogue uses max_n relu(x) = relu(max_n x) to pool before bias+relu.
"""

import numpy as np

import concourse.bacc as bacc
import concourse.bass as bass
import concourse.mybir as mybir
import concourse.tile as tile
from concourse.bass_utils import run_bass_kernel_spmd

B, T, N, F_IN = 8, 16, 1000, 16
H, D = 4, 32
HD = H * D          # 128
HL = 64
OUT = 8
NPAD = 1024
NBLK = 8
G = T

FP = mybir.dt.float32
BF = mybir.dt.bfloat16
AX = mybir.AxisListType
AF = mybir.ActivationFunctionType
OPS = mybir.AluOpType

# route per (h, J): 4=R4 (DVE mask), 5=R4G (GpSimd mask), 1=R1 (DVE TS+TT)
ROUTE = [
    [4, 2, 2, 1, 2, 2, 4, 2],
    [2, 4, 2, 2, 1, 2, 2, 4],
    [4, 2, 2, 1, 2, 2, 4, 2],
    [2, 4, 2, 2, 1, 2, 2, 4],
]

_CACHE = {}


def _build_nc():
    nc = bacc.Bacc("TRN2", target_bir_lowering=False, debug=False)

    # ---- DRAM I/O ----
    d_xt = nc.dram_tensor("x_t", [F_IN, G * NPAD], BF, kind="ExternalInput").ap()
    d_wgat = nc.dram_tensor("w_gat", [F_IN, HD], BF, kind="ExternalInput").ap()
    d_was = nc.dram_tensor("w_as", [F_IN, H], BF, kind="ExternalInput").ap()
    d_wad = nc.dram_tensor("w_ad", [F_IN, H], BF, kind="ExternalInput").ap()
    d_cnt = nc.dram_tensor("cntmask", [128, NBLK * NPAD], BF, kind="ExternalInput").ap()
    d_ones = nc.dram_tensor("ones65", [65, 128], BF, kind="ExternalInput").ap()
    d_bgat = nc.dram_tensor("b_gat", [32, H], FP, kind="ExternalInput").ap()
    d_wih = nc.dram_tensor("wih_t", [HD, 4 * HL], FP, kind="ExternalInput").ap()
    d_whh = nc.dram_tensor("whh_t", [HL, 4 * HL], FP, kind="ExternalInput").ap()
    d_bls = nc.dram_tensor("b_lstm", [HL, 4], FP, kind="ExternalInput").ap()
    d_wclf = nc.dram_tensor("wclf_t", [HL, OUT], FP, kind="ExternalInput").ap()
    d_bclf = nc.dram_tensor("b_clf", [OUT, 1], FP, kind="ExternalInput").ap()
    d_y = nc.dram_tensor("y", [OUT, 1], FP, kind="ExternalOutput").ap()

    with tile.TileContext(nc) as tc:
        with (
            tc.tile_pool(name="const", bufs=1) as cpool,
            tc.tile_pool(name="stage", bufs=3) as spool,
            tc.tile_pool(name="edense", bufs=4) as epool,
            tc.tile_pool(name="small", bufs=2) as mpool,
            tc.tile_pool(name="lstm", bufs=2) as lpool,
            tc.tile_pool(name="ps_misc", bufs=4, space="PSUM") as ps_misc,
            tc.tile_pool(name="ps_out", bufs=2, space="PSUM") as ps_out,
            tc.tile_pool(name="ps_y", bufs=1, space="PSUM") as ps_y,
        ):
            # ---- constants ----
            c_xT = cpool.tile([F_IN, G * NPAD], BF, tag="xT")
            nc.sync.dma_start(c_xT[:], d_xt)
            c_wgat = cpool.tile([F_IN, HD], BF, tag="wgat")
            nc.sync.dma_start(c_wgat[:], d_wgat)
            c_was = cpool.tile([F_IN, H], BF, tag="was")
            nc.sync.dma_start(c_was[:], d_was)
            c_wad = cpool.tile([F_IN, H], BF, tag="wad")
            nc.sync.dma_start(c_wad[:], d_wad)
            c_cnt = cpool.tile([128, NBLK * NPAD], BF, tag="cnt")
            nc.sync.dma_start(c_cnt[:], d_cnt)
            c_onesB = cpool.tile([65, 128], BF, tag="onesB")
            nc.sync.dma_start(c_onesB[:], d_ones)
            c_bgat = cpool.tile([32, H], FP, tag="bgat")
            nc.sync.dma_start(c_bgat[:], d_bgat)
            c_wih = cpool.tile([HD, 4 * HL], FP, tag="wih")
            nc.sync.dma_start(c_wih[:], d_wih)
            c_whh = cpool.tile([HL, 4 * HL], FP, tag="whh")
            nc.sync.dma_start(c_whh[:], d_whh)
            c_bls = cpool.tile([HL, 4], FP, tag="bls")
            nc.sync.dma_start(c_bls[:], d_bls)
            c_wclf = cpool.tile([HL, OUT], FP, tag="wclf")
            nc.sync.dma_start(c_wclf[:], d_wclf)
            c_bclf = cpool.tile([OUT, 1], FP, tag="bclf")
            nc.sync.dma_start(c_bclf[:], d_bclf)

            c_neg1 = cpool.tile([128, 1], FP, tag="neg1")
            nc.vector.memset(c_neg1[:], -1.0)
            c_ones32 = cpool.tile([33, 32], FP, tag="ones32")
            nc.vector.memset(c_ones32[:], 1.0)
            c_pool = cpool.tile([HD, G], FP, tag="pooled")
            c_ph = []
            for h in range(H):
                ph_tile = cpool.tile([32, G], FP, tag=f"pool{h}")
                c_ph.append(ph_tile)

            hprev0 = lpool.tile([HL, 1], FP, tag="h0")
            cprev0 = lpool.tile([HL, 1], FP, tag="c0")
            nc.vector.memset(hprev0[:], 0.0)
            nc.vector.memset(cprev0[:], 0.0)
            LST = [hprev0, cprev0]

            def emit_lstm_step(t):
                # pooled col t -> c_pool, then one LSTM step (overlaps GAT)
                for h in range(H):
                    nc.sync.dma_start(
                        c_pool[h * 32:(h + 1) * 32, t:t + 1], c_ph[h][:, t:t + 1]
                    )
                hprev, cprev = LST
                tga = []
                for gate in range(4):
                    psg = ps_misc.tile([HL, 1], FP, tag="pm")
                    nc.tensor.matmul(
                        psg[:], c_wih[:, gate * HL:(gate + 1) * HL],
                        c_pool[:, t:t + 1], start=True, stop=False,
                    )
                    nc.tensor.matmul(
                        psg[:], c_whh[:, gate * HL:(gate + 1) * HL],
                        hprev[:], start=False, stop=True,
                    )
                    tgt = lpool.tile([HL, 1], FP, tag=f"tg{gate}")
                    sc = 1.0 if gate == 2 else 0.5
                    nc.scalar.activation(
                        tgt[:], psg[:], AF.Tanh,
                        bias=c_bls[:, gate:gate + 1], scale=sc,
                    )
                    tga.append(tgt)
                ti, tf, tg_, to = tga
                v1 = lpool.tile([HL, 1], FP, tag="v1")
                nc.vector.scalar_tensor_tensor(
                    v1[:], tf[:], 1.0, cprev[:], OPS.add, OPS.mult
                )
                v2 = lpool.tile([HL, 1], FP, tag="v2")
                nc.vector.scalar_tensor_tensor(
                    v2[:], ti[:], 1.0, tg_[:], OPS.add, OPS.mult
                )
                cnew = lpool.tile([HL, 1], FP, tag="c0")
                nc.vector.scalar_tensor_tensor(
                    cnew[:], v1[:], 0.5, v2[:], OPS.mult, OPS.add
                )
                tcn = lpool.tile([HL, 1], FP, tag="tcn")
                nc.scalar.activation(tcn[:], cnew[:], AF.Tanh, scale=0.5)
                hnew = lpool.tile([HL, 1], FP, tag="h0")
                nc.vector.scalar_tensor_tensor(
                    hnew[:], to[:], 1.0, tcn[:], OPS.add, OPS.mult
                )
                LST[0], LST[1] = hnew, cnew

            PENDING = [None]
            for g in range(G):
                xg = c_xT[:, g * NPAD:(g + 1) * NPAD]   # [16, 1024] bf16

                # ---- per-g stage: a_s cols (u, rho), a_d rows (y), xpu33 ----
                pS = ps_misc.tile([128, 4 * NBLK], FP, tag="pm")
                for J in range(NBLK):
                    nc.tensor.matmul(
                        pS[:, J * 4:(J + 1) * 4],
                        xg[:, J * 128:(J + 1) * 128], c_was[:],
                        start=True, stop=True,
                    )
                c_u = spool.tile([128, 4 * NBLK], FP, tag="ucols")
                nc.scalar.activation(c_u[:], pS[:], AF.Exp, scale=1.0)
                c_rho = spool.tile([128, 4 * NBLK], FP, tag="rhocols")
                nc.scalar.activation(c_rho[:], pS[:], AF.Exp, scale=-0.8)

                y4 = spool.tile([4, NPAD], BF, tag="y4")
                for half in range(2):
                    pAd = ps_misc.tile([4, 512], FP, tag="pm")
                    nc.tensor.matmul(
                        pAd[:],
                        c_wad[:], xg[:, half * 512:(half + 1) * 512],
                        start=True, stop=True,
                    )
                    nc.scalar.activation(
                        y4[:, half * 512:(half + 1) * 512], pAd[:],
                        AF.Exp, scale=-0.8,
                    )
                # per-head base-0 row tiles for the gpsimd broadcast
                yrows = []
                for h in range(H):
                    y1h = spool.tile([1, NPAD], BF, tag=f"y1_{h}")
                    nc.sync.dma_start(y1h[:], y4[h:h + 1, :])
                    yrows.append(y1h)

                # xpu33: [128, J*132 + h*33 + (0..31 feats, 32 = u)]
                xpu33 = spool.tile([128, NBLK * 132], BF, tag="xpu33")
                for J in range(NBLK):
                    pX = ps_misc.tile([128, HD], FP, tag="pm")
                    nc.tensor.matmul(
                        pX[:], xg[:, J * 128:(J + 1) * 128], c_wgat[:],
                        start=True, stop=True,
                    )
                    base = J * 132
                    for h in range(H):
                        if J % 2 == 0:
                            nc.vector.tensor_scalar(
                                xpu33[:, base + h * 33:base + h * 33 + 32],
                                pX[:, h * 32:(h + 1) * 32],
                                c_u[:, J * 4 + h:J * 4 + h + 1], 0.0,
                                OPS.mult, OPS.add,
                            )
                        else:
                            nc.scalar.activation(
                                xpu33[:, base + h * 33:base + h * 33 + 32],
                                pX[:, h * 32:(h + 1) * 32], AF.Copy,
                                scale=c_u[:, J * 4 + h:J * 4 + h + 1],
                            )
                    nc.vector.tensor_copy(
                        xpu33[:, base:base + 132].rearrange(
                            "p (h q) -> p h q", q=33
                        )[:, :, 32:33],
                        c_u[:, J * 4:(J + 1) * 4].rearrange("p (h q) -> p h q", q=1),
                    )

                # ---- hot loop, software-pipelined one phase ahead:
                # PE stream per phase: [C-matmuls(h), yB(h)] then [tA-matmuls(h-1)]
                # so dependent matmuls trail their producers by a full phase.
                def emit_front(h):
                    oph = ps_out.tile([33, NPAD], FP, tag="oph")
                    n_mm = sum(2 if ROUTE[h][J] == 4 else 1 for J in range(NBLK))
                    mm_i = 0
                    for J in range(NBLK):
                        if ROUTE[h][J] != 4:
                            continue
                        cslice = c_cnt[:, J * NPAD:(J + 1) * NPAD]
                        lhs = xpu33[:, J * 132 + h * 33:J * 132 + h * 33 + 33]
                        for half in range(2):
                            sl = slice(half * 512, (half + 1) * 512)
                            nc.tensor.matmul(
                                oph[:, sl], lhs, cslice[:, sl],
                                start=(mm_i == 0), stop=(mm_i == n_mm - 1),
                            )
                        mm_i += 1
                    yB = epool.tile([128, NPAD], BF, tag="yB16")
                    nc.gpsimd.partition_broadcast(yB[:], yrows[h][:])
                    return (g, h, oph, yB, mm_i, n_mm, xpu33, c_rho)

                def produce(ctx):
                    g_l, h, oph, yB, mm_i, n_mm, xpu33_l, c_rho_l = ctx
                    rhss = []
                    for J in range(NBLK):
                        rt = ROUTE[h][J]
                        rho_col = c_rho_l[:, J * 4 + h:J * 4 + h + 1]
                        cslice = c_cnt[:, J * NPAD:(J + 1) * NPAD]
                        if rt == 1:
                            tAh = epool.tile([128, NPAD], BF, tag="tAh")
                            nc.vector.tensor_scalar(
                                tAh[:], yB[:], rho_col, 1.0, OPS.mult, OPS.max
                            )
                            rhs = epool.tile([128, NPAD], BF, tag="tA")
                            nc.vector.tensor_tensor(rhs[:], tAh[:], cslice, OPS.mult)
                        elif rt == 2:
                            R = epool.tile([128, NPAD], BF, tag="R")
                            nc.scalar.activation(
                                R[:], yB[:], AF.Relu, bias=c_neg1[:], scale=rho_col
                            )
                            rhs = epool.tile([128, NPAD], BF, tag="tA2")
                            nc.vector.scalar_tensor_tensor(
                                rhs[:], R[:], 1.0, cslice, OPS.add, OPS.mult
                            )
                        else:
                            R = epool.tile([128, NPAD], BF, tag="R")
                            nc.scalar.activation(
                                R[:], yB[:], AF.Relu, bias=c_neg1[:], scale=rho_col
                            )
                            rhs = epool.tile([128, NPAD], BF, tag="RC")
                            nc.vector.tensor_tensor(rhs[:], R[:], cslice, OPS.mult)
                        rhss.append(rhs)
                    return ctx + (rhss,)

                def consume(ctx):
                    g_l, h, oph, yB, mm_i, n_mm, xpu33_l, c_rho_l, rhss = ctx
                    for J in range(NBLK):
                        lhs = xpu33_l[:, J * 132 + h * 33:J * 132 + h * 33 + 33]
                        rhs = rhss[J]
                        for half in range(2):
                            sl = slice(half * 512, (half + 1) * 512)
                            nc.tensor.matmul(
                                oph[:, sl], lhs, rhs[:, half * 512:(half + 1) * 512],
                                start=(mm_i == 0), stop=(mm_i == n_mm - 1),
                            )
                        mm_i += 1
                    # epilogue
                    denr = mpool.tile([33, NPAD], FP, tag="denr")
                    nc.scalar.copy(denr[32:33, :], oph[32:33, :])
                    den32 = mpool.tile([32, 32], FP, tag="den32")
                    nc.sync.dma_start(den32[:], denr[32:33, :])
                    rec32 = mpool.tile([32, 32], FP, tag="rec32")
                    nc.vector.reciprocal(rec32[:], den32[:])
                    rech = mpool.tile([1, NPAD], FP, tag="rech")
                    nc.sync.dma_start(rech[:], rec32[:])
                    oph16 = mpool.tile([32, NPAD], BF, tag="oph16")
                    nc.scalar.copy(oph16[:], oph[0:32, :])
                    od = mpool.tile([32, NPAD], BF, tag="od")
                    for half in range(2):
                        sl = slice(half * 512, (half + 1) * 512)
                        rb = ps_misc.tile([32, 512], FP, tag="pm")
                        nc.tensor.matmul(
                            rb[:], c_ones32[0:1, :], rech[:, sl],
                            start=True, stop=True,
                        )
                        nc.vector.tensor_tensor(
                            od[:, sl], oph16[:, sl], rb[:], OPS.mult
                        )
                    trout = mpool.tile([32, 1], FP, tag="trout")
                    nc.vector.tensor_reduce(trout[:], od[:, 0:N], AX.X, OPS.max)
                    nc.vector.tensor_scalar(
                        c_ph[h][:, g_l:g_l + 1], trout[:],
                        c_bgat[:, h:h + 1], 0.0, OPS.add, OPS.max,
                    )

                for h in range(H):
                    ctx = emit_front(h)
                    if PENDING[0] is not None:
                        consume(produce(PENDING[0]))
                    PENDING[0] = ctx
                if g >= 1:
                    emit_lstm_step(g - 1)

            if PENDING[0] is not None:
                consume(produce(PENDING[0]))
                PENDING[0] = None
            emit_lstm_step(G - 1)

            ps3 = ps_misc.tile([OUT, 1], FP, tag="pm")
            nc.tensor.matmul(ps3[:], c_wclf[:], LST[0][:], start=True, stop=True)
            ysb = lpool.tile([OUT, 1], FP, tag="ysb")
            nc.vector.tensor_tensor(ysb[:], ps3[:], c_bclf[:], OPS.add)
            nc.sync.dma_start(d_y, ysb[:])

    nc.compile()
    return nc


def _host_prep(inputs):
    x = np.asarray(inputs["x"], dtype=np.float32)
    ei = np.asarray(inputs["edge_index"])
    W_gat = np.asarray(inputs["W_gat"], dtype=np.float32)
    att_src = np.asarray(inputs["att_src"], dtype=np.float32)
    att_dst = np.asarray(inputs["att_dst"], dtype=np.float32)
    b_gat = np.asarray(inputs["b_gat"], dtype=np.float32)
    W_ih = np.asarray(inputs["W_ih"], dtype=np.float32)
    W_hh = np.asarray(inputs["W_hh"], dtype=np.float32)
    b_ih = np.asarray(inputs["b_ih"], dtype=np.float32)
    b_hh = np.asarray(inputs["b_hh"], dtype=np.float32)
    W_clf = np.asarray(inputs["W_clf"], dtype=np.float32)
    b_clf = np.asarray(inputs["b_clf"], dtype=np.float32)

    bf16 = mybir.dt.np(BF)

    Wr = W_gat.reshape(F_IN, H, D)
    W_as = np.einsum("fhd,hd->fh", Wr, att_src)
    W_ad = np.einsum("fhd,hd->fh", Wr, att_dst)

    src = ei[0].astype(np.int64)
    dst = ei[1].astype(np.int64)
    Cm = np.zeros((NPAD, NPAD), dtype=np.float32)
    np.add.at(Cm, (src, dst), 1.0)
    Cm[np.arange(N), np.arange(N)] += 1.0
    Cm[NPAD - 1, N:] = 1.0
    cntmask = (
        Cm.reshape(NBLK, 128, NPAD).transpose(1, 0, 2).reshape(128, NBLK * NPAD)
    ).astype(bf16)

    xpad = np.zeros((B, T, NPAD, F_IN), dtype=np.float32)
    xpad[:, :, :N, :] = x
    # [F, T*NPAD] per core
    xtcore = [
        np.ascontiguousarray(
            xpad[b].reshape(T * NPAD, F_IN).T
        ).astype(bf16)
        for b in range(B)
    ]

    b_gates = (b_ih + b_hh).astype(np.float32)
    bls = np.zeros((HL, 4), dtype=np.float32)
    bls[:, 0] = 0.5 * b_gates[0:64]
    bls[:, 1] = 0.5 * b_gates[64:128]
    bls[:, 2] = b_gates[128:192]
    bls[:, 3] = 0.5 * b_gates[192:256]

    common = {
        "w_gat": W_gat.astype(bf16),
        "w_as": W_as.astype(bf16),
        "w_ad": W_ad.astype(bf16),
        "cntmask": cntmask,
        "ones65": np.ones((65, 128), dtype=bf16),
        "b_gat": np.ascontiguousarray(b_gat.reshape(H, 32).T),
        "wih_t": np.ascontiguousarray(W_ih.T),
        "whh_t": np.ascontiguousarray(0.5 * W_hh.T),
        "b_lstm": bls,
        "wclf_t": np.ascontiguousarray(0.5 * W_clf.T),
        "b_clf": b_clf.reshape(OUT, 1),
    }
    in_maps = []
    for b in range(B):
        m = dict(common)
        m["x_t"] = xtcore[b]
        in_maps.append(m)
    return in_maps


def kernel(**inputs):
    if "nc" not in _CACHE:
        _CACHE["nc"] = _build_nc()
    nc = _CACHE["nc"]
    in_maps = _host_prep(inputs)
    res = run_bass_kernel_spmd(nc, in_maps, core_ids=list(range(B)))
    y = np.stack([r["y"][:, 0] for r in res.results], axis=0)
    return y.astype(np.float32)


if __name__ == "__main__":
    import reference as R

    inp = R.setup_inputs()
    inp = {k: np.asarray(v) for k, v in inp.items()}
    out = kernel(**inp)
    print(out)
